# revision 30
# baseline (speedup 1.0000x reference)
"""Causal self-attention (B=2, N=2048, E=1024, H=16, D=64) on 8 TRN2 cores.

Sharding: core c -> batch b = c//4, head group g = c%4 (4 heads = 256
features per core).  Each core computes its heads' q/k/v projections,
causal attention, and a partial out-projection; the host sums the 4
partials per batch.

Per-core dataflow (feature-major "transposed" layouts throughout):
  xT [E, N] (f16)  x  wqkvT slices -> q,k as [feat, tok], v as [tok, feat]
  scoresT [ktok, qtok] = k_h^T-chunks x q_h   (PE row-tiled, 2 heads/pass)
  probsT = exp(scoresT/8) in f16 (ACT), causal triangle masked (DVE)
  attnT [feat, qtok] += v-chunk^T x probsT    (PE col-tiled, 2 heads/pass)
  denom[q] += ones^T x probsT                 (PE col-tiled M=1, 4 heads)
  attnT normalized by PE-broadcast reciprocal; out = attnT^T x woT chunks.
Causal structure skips all fully-masked k-blocks (half the attention
flops); diagonal blocks are computed on their valid q-range only.

v2 perf changes vs baseline (179.3us):
  - outproj PSUM->SBUF staging copies moved off ACT (was gating exp in
    the attention-dense j=2/3 windows) onto DVE explicitly.
  - exact RECIPROCAL (3.35us on the per-block critical chain) replaced
    by reciprocal_approx_fast on PSUM + f16 cast (~1.3us chain).
  - v-bias fold: attn bias term b_v @ wo^T is added host-side into the
    output bias, removing 8 tensor_scalar_adds from the norm chain.
  - startup DMA: xT/wq chunks interleaved and triggers spread across 4
    engine queues so the first qk units start ~3us earlier.
"""

import os
import sys
import types

import numpy as np

B, N, E, H, D = 2, 2048, 1024, 16, 64
NCORES = 8


# ---------------------------------------------------------------------------
# Environment patches (this container's walrus accepts only one sync wait per
# instruction; the image's antenv lacks the NTFF profile hook shim).
# ---------------------------------------------------------------------------

def _patch_tile_drain():
    import concourse.mybir as mybir
    import concourse.tile as tile_mod
    from concourse.vector_clock import ScopedClock

    if getattr(tile_mod.TileContext, "_drain_patched", False):
        return

    def _drain_and_barrier(self, tick_clock, wait_clock):
        nc = self.nc
        probe = nc.sync.nop()
        wait_clock.add_sem_waits(probe.ins, ScopedClock({None: tick_clock.global_clock}))
        si = probe.ins.sync_info
        waits = list(si.on_wait) if si and si.on_wait else []
        if len(waits) > 1:
            si.on_wait = waits[:1]
            for w in waits[1:]:
                extra = nc.sync.nop()
                extra.ins.sync_info = mybir.SyncInfo(on_wait=[w], on_update=[])
        nc.sync.drain()
        nc.all_engine_barrier()
        assert self.sems is not None
        popped = nc._tile_sem_poison_stack.pop()
        assert popped is self._sem_poison
        nc.clear_and_free_semaphores(list(self.sems.allocated().values()))
        nc.all_engine_barrier()

    tile_mod.TileContext._drain_and_barrier = _drain_and_barrier
    tile_mod.TileContext._drain_patched = True


def _split_sync_waits(nc, max_waits=1):
    import concourse.mybir as mybir

    cnt = 0
    for f in nc.m.functions:
        for blk in f.blocks:
            insts = blk.instructions
            new = []
            for inst in insts:
                si = inst.sync_info
                waits = list(si.on_wait) if si and si.on_wait else []
                if len(waits) > max_waits:
                    keep = waits[-max_waits:]
                    excess = waits[:-max_waits]
                    for j in range(0, len(excess), max_waits):
                        n = mybir.InstNoOp(name=f"I-ws{cnt}", ins=[], outs=[])
                        cnt += 1
                        n.engine = inst.engine
                        n.sync_info = mybir.SyncInfo(
                            on_wait=excess[j:j + max_waits], on_update=[])
                        new.append(n)
                    si.on_wait = keep
                new.append(inst)
            insts[:] = new
    return cnt


def _install_ntff_shim():
    try:
        import antenv
        if "antenv.axon_hooks" in sys.modules:
            return
        mod = types.ModuleType("antenv.axon_hooks")
        mod._hook = None
        mod.set_axon_ntff_profile_hook = lambda h: setattr(mod, "_hook", h)
        mod.get_axon_ntff_profile_hook = lambda: mod._hook
        sys.modules["antenv.axon_hooks"] = mod
        antenv.axon_hooks = mod
        from trn_agent_boot.trn_boot import _ntff_profile_via_ctypes
        mod._hook = _ntff_profile_via_ctypes("/opt/axon/libaxon_pjrt.so")
    except Exception:
        pass


# ---------------------------------------------------------------------------
# Device program (identical on all 8 cores)
# ---------------------------------------------------------------------------

def _build_nc():
    import concourse.bass as bass
    import concourse.mybir as mybir
    import concourse.tile as tile

    _patch_tile_drain()

    f32 = mybir.dt.float32
    f16 = mybir.dt.float16
    AF = mybir.ActivationFunctionType

    nc = bass.Bass("TRN2", target_bir_lowering=False, debug=False)

    xT = nc.dram_tensor("xT", [E, N], f16, kind="ExternalInput")
    wqkvT = nc.dram_tensor("wqkvT", [E, 768], f16, kind="ExternalInput")
    woT = nc.dram_tensor("woT", [256, E], f16, kind="ExternalInput")
    bqkv = nc.dram_tensor("bqkv", [128, 4], f32, kind="ExternalInput")
    tri = nc.dram_tensor("tri", [128, 2, 128], f16, kind="ExternalInput")
    outp = nc.dram_tensor("outp", [N, E], f16, kind="ExternalOutput")

    NB = N // 512          # 4 token blocks of 512
    NT = N // 128          # 16 token tiles of 128
    NE = E // 128          # 8 contraction chunks
    SCALE = float(D) ** -0.5

    with nc.allow_low_precision(reason="fp16 matmul pipeline"), \
            tile.TileContext(nc) as tc:
        with tc.tile_pool(name="const", bufs=1) as constp, \
                tc.tile_pool(name="qk", bufs=1) as qkp, \
                tc.tile_pool(name="probs", bufs=6) as pbp, \
                tc.tile_pool(name="misc", bufs=2) as miscp, \
                tc.tile_pool(name="stage", bufs=4) as stp, \
                tc.tile_pool(name="mm", bufs=1, space="PSUM") as mmp, \
                tc.tile_pool(name="sc", bufs=2, space="PSUM") as scp, \
                tc.tile_pool(name="pv", bufs=1, space="PSUM") as pvp, \
                tc.tile_pool(name="den", bufs=1, space="PSUM") as denp:

            xT_sb = constp.tile([128, NE, N], f16, tag="xT")
            wq_sb = constp.tile([128, NE, 768], f16, tag="wq")
            wo_sb = constp.tile([128, 2, E], f16, tag="wo")
            bias_sb = constp.tile([128, 4, 1], f32, tag="bias")
            tri_sb = constp.tile([128, 2, 128], f16, tag="tri")
            ones_sb = constp.tile([128, 64], f16, tag="ones")
            q_sb = qkp.tile([128, 2, N], f16, tag="q")
            k_sb = qkp.tile([128, 2, N], f16, tag="k")
            vt_sb = qkp.tile([128, NT, 256], f16, tag="vt")
            at_sb = qkp.tile([128, 2, N], f16, tag="at")

            nc.vector.memset(ones_sb[:], 1.0)
            # Startup DMA: interleave xT block-0 chunks with wq chunks and
            # spread triggers over 4 engine queues so the first qk units
            # (which consume chunk e as it lands) start as early as possible.
            # Tiny constants (bias, tri) go FIRST as single descriptors: the
            # first qk unit's bias add must not wait for bulk traffic, and
            # only two trigger slots (~1.2us) delay the wq stream.
            nc.scalar.dma_start(bias_sb[:, :, 0], bqkv.ap())
            nc.scalar.dma_start(tri_sb[:], tri.ap())
            for i in range(NE):
                nc.sync.dma_start(
                    xT_sb[:, i, 0:512],
                    xT.ap()[i * 128:(i + 1) * 128, 0:512])
                nc.scalar.dma_start(
                    wq_sb[:, i, :], wqkvT.ap()[i * 128:(i + 1) * 128, :])
            for i in range(NE):
                (nc.sync if i % 2 == 0 else nc.scalar).dma_start(
                    xT_sb[:, i, 512:1024],
                    xT.ap()[i * 128:(i + 1) * 128, 512:1024])
            for i in range(NE):
                (nc.sync if i % 2 == 0 else nc.scalar).dma_start(
                    xT_sb[:, i, 1024:2048],
                    xT.ap()[i * 128:(i + 1) * 128, 1024:2048])
            for i in range(2):
                nc.scalar.dma_start(wo_sb[:, i, :], woT.ap()[i * 128:(i + 1) * 128, :])

            # -- filler work units (emitted interleaved into attention) ----
            _qk_open = {}

            def emit_qk_unit(ft, nb, pool=None, tag="mm", on_act=False,
                             half=None):
                # q/k feature tile ft (0,1=q; 2,3=k), token block nb.
                # on_act: drain the PSUM bank via ACT instead of DVE, for
                # fillers adjacent to the block-boundary reciprocal chain
                # (DVE strict FIFO would park the bank behind the recips).
                # half=0/1: emit only e-chunks 0-3 / 4-7 (finer filler
                # pacing); half 0 allocates the bank, half 1 finishes+drains.
                if half == 0:
                    ps = (pool or mmp).tile([128, 512], f32, tag=tag,
                                            name=f"qk_{ft}_{nb}")
                    _qk_open[(ft, nb)] = ps
                    erange = range(0, NE // 2)
                elif half == 1:
                    ps = _qk_open.pop((ft, nb))
                    erange = range(NE // 2, NE)
                else:
                    ps = (pool or mmp).tile([128, 512], f32, tag=tag,
                                            name=f"qk_{ft}_{nb}")
                    erange = range(NE)
                for e in erange:
                    nc.tensor.matmul(
                        ps[:], wq_sb[:, e, ft * 128:(ft + 1) * 128],
                        xT_sb[:, e, nb * 512:(nb + 1) * 512],
                        start=(e == 0), stop=(e == NE - 1))
                if half == 0:
                    return
                dest = (q_sb if ft < 2 else k_sb)[:, ft % 2,
                                                  nb * 512:(nb + 1) * 512]
                if on_act:
                    nc.scalar.add(dest, ps[:], bias_sb[:, ft, :])
                else:
                    nc.vector.tensor_scalar_add(dest, ps[:], bias_sb[:, ft, :])

            def emit_v_unit(tt, pool=None, tag="mm", on_act=False):
                ps = (pool or mmp).tile([128, 256], f32, tag=tag,
                                        name=f"v_{tt}")
                for e in range(NE):
                    nc.tensor.matmul(
                        ps[:], xT_sb[:, e, tt * 128:(tt + 1) * 128],
                        wq_sb[:, e, 512:768],
                        start=(e == 0), stop=(e == NE - 1))
                if on_act:
                    nc.scalar.copy(vt_sb[:, tt, :], ps[:])
                else:
                    nc.vector.tensor_copy(vt_sb[:, tt, :], ps[:])

            def emit_p3_unit(tt, on_act=False):
                for n2 in range(2):
                    ps = mmp.tile([128, 512], f32, tag="mm",
                                  name=f"p3_{tt}_{n2}")
                    for fp in range(2):
                        nc.tensor.matmul(
                            ps[:],
                            at_sb[:, fp, tt * 128:(tt + 1) * 128],
                            wo_sb[:, fp, n2 * 512:(n2 + 1) * 512],
                            start=(fp == 0), stop=(fp == 1))
                    st = stp.tile([128, 512], f16, tag="st",
                                  name=f"st_{tt}_{n2}")
                    if on_act:
                        nc.scalar.copy(st[:], ps[:])
                    else:
                        nc.vector.tensor_copy(st[:], ps[:])
                    nc.sync.dma_start(
                        outp.ap()[tt * 128:(tt + 1) * 128,
                                  n2 * 512:(n2 + 1) * 512], st[:])

            def emit_norm(item, p, c0, c1):
                # PE broadcast of 1/denom + normalize into attnT for head
                # pair p, qtok columns [c0, c1) of the block.  The two
                # matmuls hit disjoint partition halves of one PSUM bank, so
                # they run concurrently without a bank collision; the DVE
                # multiply latency is covered by fillers emitted in between
                # successive emit_norm calls.
                jj, araw, rec = item
                w = c1 - c0
                cs = slice(c0, c1)
                js = slice(512 * jj + c0, 512 * jj + c1)
                bc = mmp.tile([128, 512], f32, tag="mm",
                              name=f"bc_{jj}_{p}_{c0}")
                for hh in (0, 1):
                    h = 2 * p + hh
                    nc.tensor.matmul(
                        bc[64 * hh:64 * hh + 64, 0:w],
                        ones_sb[32 * h:32 * h + 1, 0:64],
                        rec[32 * h:32 * h + 1, cs],
                        start=True, stop=True,
                        tile_position=(32 * h, 64 * hh))
                nc.vector.tensor_mul(at_sb[:, p, js],
                                     araw[p][:, cs], bc[:, 0:w])

            # -- p1 block 0 up front (attention j=0 needs it).  Attention
            # PSUM banks are idle here; round-robin the units through them
            # so consecutive units don't serialize on one bank's DVE drain.
            upools = [(mmp, "mm"), (pvp, "pv0"), (pvp, "pv1"), (denp, "den")]
            for u, ft in enumerate(range(4)):
                emit_qk_unit(ft, 0, *upools[u % 4])
            for u, tt in enumerate(range(4)):
                emit_v_unit(tt, *upools[u % 4])

            # -- attention blocks: depth-2 software pipeline ---------------
            # scores/exp for k-block ik+2 are emitted while pv/den of block
            # ik execute, so ACT (exp) and PE overlap instead of ping-pong.
            pending = None
            for j in range(NB):
                if j == 0:
                    fillers = []
                    for ft in range(4):
                        fillers += [(emit_qk_unit, (ft, 1, None, "mm", False, 0)),
                                    (emit_qk_unit, (ft, 1, None, "mm", False, 1))]
                    fillers += [(emit_v_unit, (tt,)) for tt in range(4, 8)]
                elif j == 1:
                    fillers = []
                    for ft in range(4):
                        fillers += [(emit_qk_unit, (ft, 2, None, "mm", False, 0)),
                                    (emit_qk_unit, (ft, 2, None, "mm", False, 1))]
                    fillers += [(emit_v_unit, (tt,)) for tt in range(8, 12)]
                elif j == 2:
                    fillers = []
                    for ft in range(4):
                        fillers += [(emit_qk_unit, (ft, 3, None, "mm", False, 0)),
                                    (emit_qk_unit, (ft, 3, None, "mm", False, 1))]
                    fillers += [(emit_p3_unit, (tt,)) for tt in range(2)]
                else:
                    fillers = [(emit_v_unit, (tt,)) for tt in range(12, 16)]
                    fillers += [(emit_p3_unit, (tt,)) for tt in range(2, 12)]
                if pending is not None:
                    fillers.insert(min(3, len(fillers)),
                                   (emit_norm, (pending[:3], 0, 0, 512)))
                    fillers.insert(min(5, len(fillers)),
                                   (emit_norm, (pending[:3], 1, 0, 512)))
                    pending = None
                nf = len(fillers)
                pv_ps = [pvp.tile([128, 512], f32, tag=f"pv{p}",
                                  name=f"pv{p}_{j}") for p in (0, 1)]
                den_ps = denp.tile([128, 512], f32, tag="den",
                                   name=f"den_{j}")
                nk = 4 * (j + 1)

                def emit_scores(ik, j=j):
                    r = ik - 4 * j
                    qoff = 128 * r if r > 0 else 0
                    qs = slice(512 * j + qoff, 512 * (j + 1))
                    pbs = []
                    for p in (0, 1):
                        sc = scp.tile([128, 2, 512], f32, tag="sc",
                                      name=f"sc_{j}_{ik}_{p}")
                        for hh in (0, 1):
                            dsl = slice(64 * hh, 64 * hh + 64)
                            nc.tensor.matmul(
                                sc[:, hh, qoff:512],
                                k_sb[dsl, p, ik * 128:(ik + 1) * 128],
                                q_sb[dsl, p, qs],
                                start=True, stop=True)
                        pb = pbp.tile([128, 2, 512], f16, tag="pb",
                                      name=f"pb_{j}_{ik}_{p}")
                        nc.scalar.activation(pb[:, :, qoff:512],
                                             sc[:, :, qoff:512],
                                             AF.Exp, scale=SCALE)
                        if r >= 0:
                            nc.gpsimd.tensor_mul(
                                pb[:, :, qoff:qoff + 128],
                                pb[:, :, qoff:qoff + 128], tri_sb[:])
                        pbs.append(pb)
                    return pbs

                stage = {0: emit_scores(0)}
                if nk > 1:
                    stage[1] = emit_scores(1)
                fdone = 0
                den_put = []
                for ik in range(nk):
                    r = ik - 4 * j
                    qoff = 128 * r if r > 0 else 0
                    first, last = ik == 0, ik == nk - 1
                    pbs = stage.pop(ik)
                    den_put.append((ik, qoff, pbs, first, last))
                    if ik % 2 == 1 or last:
                        for dik, dqoff, dpbs, dfirst, dlast in den_put:
                            for p in (0, 1):
                                for hh in (0, 1):
                                    h = 2 * p + hh
                                    nc.tensor.matmul(
                                        pv_ps[p][64 * hh:64 * hh + 64,
                                                 dqoff:512],
                                        vt_sb[:, dik, 64 * h:64 * h + 64],
                                        dpbs[p][:, hh, dqoff:512],
                                        start=dfirst, stop=dlast,
                                        tile_position=(0, 64 * hh),
                                        skip_group_check=True)
                    if ik % 2 == 1 or last:
                        for dik, dqoff, dpbs, dfirst, dlast in den_put:
                            for h in range(4):
                                nc.tensor.matmul(
                                    den_ps[32 * h:32 * h + 1, dqoff:512],
                                    ones_sb[:, 0:1],
                                    dpbs[h // 2][:, h % 2, dqoff:512],
                                    start=dfirst, stop=dlast,
                                    tile_position=(0, 32 * h),
                                    skip_group_check=True)
                        den_put = []
                    want = ((ik + 1) * nf) // nk
                    while fdone < want:
                        fn, args = fillers[fdone]
                        fn(*args)
                        fdone += 1
                    if ik + 2 < nk:
                        stage[ik + 2] = emit_scores(ik + 2)
                araw = [miscp.tile([128, 512], f32, tag=f"araw{p}",
                                   name=f"araw{p}_{j}") for p in (0, 1)]
                # 1/den via the exp/ln activation table (same table set
                # as Exp -> no ACT_TABLE_LOAD): rec = exp(-ln(den)).  Keeps
                # the block-boundary chain off DVE's strict FIFO and frees
                # the den bank ~3us earlier than the exact DVE reciprocal.
                den_ln = miscp.tile([128, 512], f32, tag="densb",
                                    name=f"densb_{j}")
                rec = miscp.tile([128, 512], f16, tag="rec", name=f"rec_{j}")
                if j < NB - 1:
                    for p in (0, 1):
                        nc.vector.tensor_copy(araw[p][:], pv_ps[p][:])
                    nc.scalar.activation(den_ln[0:97, :], den_ps[0:97, :],
                                         AF.Ln)
                    nc.scalar.activation(rec[0:97, :], den_ln[0:97, :],
                                         AF.Exp, scale=-1.0)
                pending = (j, araw, rec, pv_ps, den_ps)

            # -- tail: final normalize + last output tiles.  Interleaved by
            # column half: tt 12-13 only need the first 256 qtok of block 3,
            # so their out-projection runs while half 1 normalizes. --------
            def tail_p3(tt):
                for n2 in range(2):
                    ps = scp.tile([128, 512], f32, tag="sc",
                                  name=f"p3t_{tt}_{n2}")
                    for fp in range(2):
                        nc.tensor.matmul(
                            ps[:],
                            at_sb[:, fp, tt * 128:(tt + 1) * 128],
                            wo_sb[:, fp, n2 * 512:(n2 + 1) * 512],
                            start=(fp == 0), stop=(fp == 1))
                    st = stp.tile([128, 512], f16, tag="st",
                                  name=f"stt_{tt}_{n2}")
                    if n2 == 0:
                        nc.vector.tensor_copy(st[:], ps[:])
                    else:
                        nc.scalar.copy(st[:], ps[:])
                    nc.sync.dma_start(
                        outp.ap()[tt * 128:(tt + 1) * 128,
                                  n2 * 512:(n2 + 1) * 512], st[:])

            jj, araw3, rec3, pv3, den3 = pending
            den_ln3 = miscp.tile([128, 512], f32, tag="densb", name="densb3b")
            item3 = (jj, araw3, rec3)
            for qq in range(4):
                cq = slice(128 * qq, 128 * qq + 128)
                nc.vector.tensor_copy(araw3[0][:, cq], pv3[0][:, cq])
                nc.vector.tensor_copy(araw3[1][:, cq], pv3[1][:, cq])
                nc.scalar.activation(den_ln3[0:97, cq], den3[0:97, cq], AF.Ln)
                nc.scalar.activation(rec3[0:97, cq], den_ln3[0:97, cq],
                                     AF.Exp, scale=-1.0)
                emit_norm(item3, 0, 128 * qq, 128 * qq + 128)
                emit_norm(item3, 1, 128 * qq, 128 * qq + 128)
                tail_p3(12 + qq)

    _split_sync_waits(nc)
    return nc


_NC = None


def _get_nc():
    global _NC
    if _NC is None:
        _NC = _build_nc()
    return _NC


# ---------------------------------------------------------------------------
# Host entry point
# ---------------------------------------------------------------------------

def kernel(x, qkv_w, qkv_b, out_w, out_b):
    from concourse.bass_utils import run_bass_kernel_spmd

    trace_dir = os.environ.get("BASS_KERNEL_TRACE_DIR")
    if trace_dir:
        _install_ntff_shim()

    nc = _get_nc()

    x = np.asarray(x, np.float32)
    qkv_w = np.asarray(qkv_w, np.float32)
    qkv_b = np.asarray(qkv_b, np.float32)
    out_w = np.asarray(out_w, np.float32)
    out_b = np.asarray(out_b, np.float32)

    tri_np = np.broadcast_to(np.triu(np.ones((128, 128), np.float16))[:, None, :],
        (128, 2, 128)).copy()
    in_maps = []
    for c in range(NCORES):
        b, g = divmod(c, 4)
        fs = slice(256 * g, 256 * g + 256)
        wqkvT = np.ascontiguousarray(
            np.concatenate([qkv_w[0 * E:1 * E][fs],
                            qkv_w[1 * E:2 * E][fs],
                            qkv_w[2 * E:3 * E][fs]], axis=0).T)
        bq = np.concatenate([qkv_b[0 * E:1 * E][fs],
                             qkv_b[1 * E:2 * E][fs]])[:, None]
        in_maps.append({
            "xT": np.ascontiguousarray(x[b].T).astype(np.float16),
            "wqkvT": wqkvT.astype(np.float16),
            "woT": np.ascontiguousarray(out_w[:, fs].T).astype(np.float16),
            "bqkv": np.ascontiguousarray(bq),
            "tri": tri_np,
        })

    kwargs = {}
    if trace_dir:
        kwargs = {"trace": True, "tmpdir": trace_dir}
    res = run_bass_kernel_spmd(nc, in_maps, core_ids=list(range(NCORES)), **kwargs)
    if trace_dir and res.exec_time_ns is not None:
        print(f"HW exec time: {res.exec_time_ns} ns")

    out = np.zeros((B, N, E), np.float32)
    for c in range(NCORES):
        out[c // 4] += res.results[c]["outp"].astype(np.float32)
    # v-bias is not applied on-device; its out-projection image is a constant
    # vector folded into the output bias here: (attn + b_v) @ W_o^T
    #   = attn @ W_o^T + b_v @ W_o^T.
    out += (out_b + qkv_b[2 * E:] @ out_w.T)[None, None, :]
    return out


# revision 31
# speedup vs baseline: 1.0281x; 1.0281x over previous
"""Causal self-attention (B=2, N=2048, E=1024, H=16, D=64) on 8 TRN2 cores.

Sharding: core c -> batch b = c//4, head group g = c%4 (4 heads = 256
features per core).  Each core computes its heads' q/k/v projections,
causal attention, and a partial out-projection; the host sums the 4
partials per batch.

Per-core dataflow (feature-major "transposed" layouts throughout):
  xT [E, N] (f16)  x  wqkvT slices -> q,k as [feat, tok], v as [tok, feat]
  scoresT [ktok, qtok] = k_h^T-chunks x q_h   (PE row-tiled, 2 heads/pass)
  probsT = exp(scoresT/8) in f16 (ACT), causal triangle masked (DVE)
  attnT [feat, qtok] += v-chunk^T x probsT    (PE col-tiled, 2 heads/pass)
  denom[q] += ones^T x probsT                 (PE col-tiled M=1, 4 heads)
  attnT normalized by PE-broadcast reciprocal; out = attnT^T x woT chunks.
Causal structure skips all fully-masked k-blocks (half the attention
flops); diagonal blocks are computed on their valid q-range only.

v2 perf changes vs baseline (179.3us):
  - outproj PSUM->SBUF staging copies moved off ACT (was gating exp in
    the attention-dense j=2/3 windows) onto DVE explicitly.
  - exact RECIPROCAL (3.35us on the per-block critical chain) replaced
    by reciprocal_approx_fast on PSUM + f16 cast (~1.3us chain).
  - v-bias fold: attn bias term b_v @ wo^T is added host-side into the
    output bias, removing 8 tensor_scalar_adds from the norm chain.
  - startup DMA: xT/wq chunks interleaved and triggers spread across 4
    engine queues so the first qk units start ~3us earlier.
"""

import os
import sys
import types

import numpy as np

B, N, E, H, D = 2, 2048, 1024, 16, 64
NCORES = 8


# ---------------------------------------------------------------------------
# Environment patches (this container's walrus accepts only one sync wait per
# instruction; the image's antenv lacks the NTFF profile hook shim).
# ---------------------------------------------------------------------------

def _patch_tile_drain():
    import concourse.mybir as mybir
    import concourse.tile as tile_mod
    from concourse.vector_clock import ScopedClock

    if getattr(tile_mod.TileContext, "_drain_patched", False):
        return

    def _drain_and_barrier(self, tick_clock, wait_clock):
        nc = self.nc
        probe = nc.sync.nop()
        wait_clock.add_sem_waits(probe.ins, ScopedClock({None: tick_clock.global_clock}))
        si = probe.ins.sync_info
        waits = list(si.on_wait) if si and si.on_wait else []
        if len(waits) > 1:
            si.on_wait = waits[:1]
            for w in waits[1:]:
                extra = nc.sync.nop()
                extra.ins.sync_info = mybir.SyncInfo(on_wait=[w], on_update=[])
        nc.sync.drain()
        nc.all_engine_barrier()
        assert self.sems is not None
        popped = nc._tile_sem_poison_stack.pop()
        assert popped is self._sem_poison
        nc.clear_and_free_semaphores(list(self.sems.allocated().values()))
        nc.all_engine_barrier()

    tile_mod.TileContext._drain_and_barrier = _drain_and_barrier
    tile_mod.TileContext._drain_patched = True


def _split_sync_waits(nc, max_waits=1):
    import concourse.mybir as mybir

    cnt = 0
    for f in nc.m.functions:
        for blk in f.blocks:
            insts = blk.instructions
            new = []
            for inst in insts:
                si = inst.sync_info
                waits = list(si.on_wait) if si and si.on_wait else []
                if len(waits) > max_waits:
                    keep = waits[-max_waits:]
                    excess = waits[:-max_waits]
                    for j in range(0, len(excess), max_waits):
                        n = mybir.InstNoOp(name=f"I-ws{cnt}", ins=[], outs=[])
                        cnt += 1
                        n.engine = inst.engine
                        n.sync_info = mybir.SyncInfo(
                            on_wait=excess[j:j + max_waits], on_update=[])
                        new.append(n)
                    si.on_wait = keep
                new.append(inst)
            insts[:] = new
    return cnt


def _install_ntff_shim():
    try:
        import antenv
        if "antenv.axon_hooks" in sys.modules:
            return
        mod = types.ModuleType("antenv.axon_hooks")
        mod._hook = None
        mod.set_axon_ntff_profile_hook = lambda h: setattr(mod, "_hook", h)
        mod.get_axon_ntff_profile_hook = lambda: mod._hook
        sys.modules["antenv.axon_hooks"] = mod
        antenv.axon_hooks = mod
        from trn_agent_boot.trn_boot import _ntff_profile_via_ctypes
        mod._hook = _ntff_profile_via_ctypes("/opt/axon/libaxon_pjrt.so")
    except Exception:
        pass


# ---------------------------------------------------------------------------
# Device program (identical on all 8 cores)
# ---------------------------------------------------------------------------

def _build_nc():
    import concourse.bass as bass
    import concourse.mybir as mybir
    import concourse.tile as tile

    _patch_tile_drain()

    f32 = mybir.dt.float32
    f16 = mybir.dt.float16
    AF = mybir.ActivationFunctionType

    nc = bass.Bass("TRN2", target_bir_lowering=False, debug=False)

    xT = nc.dram_tensor("xT", [E, N], f16, kind="ExternalInput")
    wqkvT = nc.dram_tensor("wqkvT", [E, 768], f16, kind="ExternalInput")
    woT = nc.dram_tensor("woT", [256, E], f16, kind="ExternalInput")
    bqkv = nc.dram_tensor("bqkv", [128, 4], f32, kind="ExternalInput")
    tri = nc.dram_tensor("tri", [128, 2, 128], f16, kind="ExternalInput")
    outp = nc.dram_tensor("outp", [N, E], f16, kind="ExternalOutput")

    NB = N // 512          # 4 token blocks of 512
    NT = N // 128          # 16 token tiles of 128
    NE = E // 128          # 8 contraction chunks
    SCALE = float(D) ** -0.5

    with nc.allow_low_precision(reason="fp16 matmul pipeline"), \
            tile.TileContext(nc) as tc:
        with tc.tile_pool(name="const", bufs=1) as constp, \
                tc.tile_pool(name="qk", bufs=1) as qkp, \
                tc.tile_pool(name="probs", bufs=6) as pbp, \
                tc.tile_pool(name="misc", bufs=2) as miscp, \
                tc.tile_pool(name="stage", bufs=4) as stp, \
                tc.tile_pool(name="mm", bufs=1, space="PSUM") as mmp, \
                tc.tile_pool(name="sc", bufs=2, space="PSUM") as scp, \
                tc.tile_pool(name="pv", bufs=1, space="PSUM") as pvp, \
                tc.tile_pool(name="den", bufs=1, space="PSUM") as denp:

            xT_sb = constp.tile([128, NE, N], f16, tag="xT")
            wq_sb = constp.tile([128, NE, 768], f16, tag="wq")
            wo_sb = constp.tile([128, 2, E], f16, tag="wo")
            bias_sb = constp.tile([128, 4, 1], f32, tag="bias")
            tri_sb = constp.tile([128, 2, 128], f16, tag="tri")
            ones_sb = constp.tile([128, 64], f16, tag="ones")
            q_sb = qkp.tile([128, 2, N], f16, tag="q")
            k_sb = qkp.tile([128, 2, N], f16, tag="k")
            vt_sb = qkp.tile([128, NT, 256], f16, tag="vt")
            at_sb = qkp.tile([128, 2, N], f16, tag="at")

            nc.vector.memset(ones_sb[:], 1.0)
            # Startup DMA: interleave xT block-0 chunks with wq chunks and
            # spread triggers over 4 engine queues so the first qk units
            # (which consume chunk e as it lands) start as early as possible.
            # Tiny constants (bias, tri) go FIRST as single descriptors: the
            # first qk unit's bias add must not wait for bulk traffic, and
            # only two trigger slots (~1.2us) delay the wq stream.
            nc.scalar.dma_start(bias_sb[:, :, 0], bqkv.ap())
            nc.scalar.dma_start(tri_sb[:], tri.ap())
            for i in range(NE):
                nc.sync.dma_start(
                    xT_sb[:, i, 0:512],
                    xT.ap()[i * 128:(i + 1) * 128, 0:512])
                nc.scalar.dma_start(
                    wq_sb[:, i, :], wqkvT.ap()[i * 128:(i + 1) * 128, :])
            for i in range(NE):
                (nc.sync if i % 2 == 0 else nc.scalar).dma_start(
                    xT_sb[:, i, 512:1024],
                    xT.ap()[i * 128:(i + 1) * 128, 512:1024])
            for i in range(NE):
                (nc.sync if i % 2 == 0 else nc.scalar).dma_start(
                    xT_sb[:, i, 1024:2048],
                    xT.ap()[i * 128:(i + 1) * 128, 1024:2048])
            for i in range(2):
                nc.scalar.dma_start(wo_sb[:, i, :], woT.ap()[i * 128:(i + 1) * 128, :])

            # -- filler work units (emitted interleaved into attention) ----
            _qk_open = {}

            def emit_qk_unit(ft, nb, pool=None, tag="mm", on_act=False,
                             half=None):
                # q/k feature tile ft (0,1=q; 2,3=k), token block nb.
                # on_act: drain the PSUM bank via ACT instead of DVE, for
                # fillers adjacent to the block-boundary reciprocal chain
                # (DVE strict FIFO would park the bank behind the recips).
                # half=0/1: emit only e-chunks 0-3 / 4-7 (finer filler
                # pacing); half 0 allocates the bank, half 1 finishes+drains.
                if half == 0:
                    ps = (pool or mmp).tile([128, 512], f32, tag=tag,
                                            name=f"qk_{ft}_{nb}")
                    _qk_open[(ft, nb)] = ps
                    erange = range(0, NE // 2)
                elif half == 1:
                    ps = _qk_open.pop((ft, nb))
                    erange = range(NE // 2, NE)
                else:
                    ps = (pool or mmp).tile([128, 512], f32, tag=tag,
                                            name=f"qk_{ft}_{nb}")
                    erange = range(NE)
                for e in erange:
                    nc.tensor.matmul(
                        ps[:], wq_sb[:, e, ft * 128:(ft + 1) * 128],
                        xT_sb[:, e, nb * 512:(nb + 1) * 512],
                        start=(e == 0), stop=(e == NE - 1))
                if half == 0:
                    return
                dest = (q_sb if ft < 2 else k_sb)[:, ft % 2,
                                                  nb * 512:(nb + 1) * 512]
                if on_act:
                    nc.scalar.add(dest, ps[:], bias_sb[:, ft, :])
                else:
                    nc.vector.tensor_scalar_add(dest, ps[:], bias_sb[:, ft, :])

            def emit_v_unit(tt, pool=None, tag="mm", on_act=False):
                ps = (pool or mmp).tile([128, 256], f32, tag=tag,
                                        name=f"v_{tt}")
                for e in range(NE):
                    nc.tensor.matmul(
                        ps[:], xT_sb[:, e, tt * 128:(tt + 1) * 128],
                        wq_sb[:, e, 512:768],
                        start=(e == 0), stop=(e == NE - 1))
                if on_act:
                    nc.scalar.copy(vt_sb[:, tt, :], ps[:])
                else:
                    nc.vector.tensor_copy(vt_sb[:, tt, :], ps[:])

            def emit_p3_unit(tt, on_act=False):
                for n2 in range(2):
                    ps = mmp.tile([128, 512], f32, tag="mm",
                                  name=f"p3_{tt}_{n2}")
                    for fp in range(2):
                        nc.tensor.matmul(
                            ps[:],
                            at_sb[:, fp, tt * 128:(tt + 1) * 128],
                            wo_sb[:, fp, n2 * 512:(n2 + 1) * 512],
                            start=(fp == 0), stop=(fp == 1))
                    st = stp.tile([128, 512], f16, tag="st",
                                  name=f"st_{tt}_{n2}")
                    if on_act:
                        nc.scalar.copy(st[:], ps[:])
                    else:
                        nc.vector.tensor_copy(st[:], ps[:])
                    nc.sync.dma_start(
                        outp.ap()[tt * 128:(tt + 1) * 128,
                                  n2 * 512:(n2 + 1) * 512], st[:])

            def emit_norm(item, p, c0, c1):
                # PE broadcast of 1/denom + normalize into attnT for head
                # pair p, qtok columns [c0, c1) of the block.  The two
                # matmuls hit disjoint partition halves of one PSUM bank, so
                # they run concurrently without a bank collision; the DVE
                # multiply latency is covered by fillers emitted in between
                # successive emit_norm calls.
                jj, araw, rec = item
                w = c1 - c0
                cs = slice(c0, c1)
                js = slice(512 * jj + c0, 512 * jj + c1)
                bc = mmp.tile([128, 512], f32, tag="mm",
                              name=f"bc_{jj}_{p}_{c0}")
                for hh in (0, 1):
                    h = 2 * p + hh
                    nc.tensor.matmul(
                        bc[64 * hh:64 * hh + 64, 0:w],
                        ones_sb[32 * h:32 * h + 1, 0:64],
                        rec[32 * h:32 * h + 1, cs],
                        start=True, stop=True,
                        tile_position=(32 * h, 64 * hh))
                nc.vector.tensor_mul(at_sb[:, p, js],
                                     araw[p][:, cs], bc[:, 0:w])

            # -- p1 block 0 up front (attention j=0 needs it).  Attention
            # PSUM banks are idle here; round-robin the units through them
            # so consecutive units don't serialize on one bank's DVE drain.
            upools = [(mmp, "mm"), (pvp, "pv0"), (pvp, "pv1"), (denp, "den")]
            for u, ft in enumerate(range(4)):
                emit_qk_unit(ft, 0, *upools[u % 4])
            for u, tt in enumerate(range(4)):
                emit_v_unit(tt, *upools[u % 4])

            # -- attention blocks: depth-2 software pipeline ---------------
            # scores/exp for k-block ik+2 are emitted while pv/den of block
            # ik execute, so ACT (exp) and PE overlap instead of ping-pong.
            pending = None
            for j in range(NB):
                if j == 0:
                    fillers = []
                    for ft in range(4):
                        fillers += [(emit_qk_unit, (ft, 1, None, "mm", False, 0)),
                                    (emit_qk_unit, (ft, 1, None, "mm", False, 1))]
                    fillers += [(emit_v_unit, (tt,)) for tt in range(4, 8)]
                elif j == 1:
                    fillers = []
                    for ft in range(4):
                        fillers += [(emit_qk_unit, (ft, 2, None, "mm", False, 0)),
                                    (emit_qk_unit, (ft, 2, None, "mm", False, 1))]
                    fillers += [(emit_v_unit, (tt,)) for tt in range(8, 12)]
                elif j == 2:
                    fillers = []
                    for ft in range(4):
                        fillers += [(emit_qk_unit, (ft, 3, None, "mm", False, 0)),
                                    (emit_qk_unit, (ft, 3, None, "mm", False, 1))]
                    fillers += [(emit_p3_unit, (tt,)) for tt in range(2)]
                else:
                    fillers = [(emit_v_unit, (tt,)) for tt in range(12, 16)]
                    fillers += [(emit_p3_unit, (tt,)) for tt in range(2, 12)]
                if pending is not None:
                    fillers.insert(min(3, len(fillers)),
                                   (emit_norm, (pending[:3], 0, 0, 512)))
                    fillers.insert(min(5, len(fillers)),
                                   (emit_norm, (pending[:3], 1, 0, 512)))
                    pending = None
                nf = len(fillers)
                pv_ps = [pvp.tile([128, 512], f32, tag=f"pv{p}",
                                  name=f"pv{p}_{j}") for p in (0, 1)]
                den_ps = denp.tile([128, 512], f32, tag="den",
                                   name=f"den_{j}")
                nk = 4 * (j + 1)

                def emit_scores(ik, j=j):
                    r = ik - 4 * j
                    qoff = 128 * r if r > 0 else 0
                    qs = slice(512 * j + qoff, 512 * (j + 1))
                    pbs = []
                    for p in (0, 1):
                        sc = scp.tile([128, 2, 512], f32, tag="sc",
                                      name=f"sc_{j}_{ik}_{p}")
                        for hh in (0, 1):
                            dsl = slice(64 * hh, 64 * hh + 64)
                            nc.tensor.matmul(
                                sc[:, hh, qoff:512],
                                k_sb[dsl, p, ik * 128:(ik + 1) * 128],
                                q_sb[dsl, p, qs],
                                start=True, stop=True)
                        pb = pbp.tile([128, 2, 512], f16, tag="pb",
                                      name=f"pb_{j}_{ik}_{p}")
                        nc.scalar.activation(pb[:, :, qoff:512],
                                             sc[:, :, qoff:512],
                                             AF.Exp, scale=SCALE)
                        if r >= 0:
                            nc.gpsimd.tensor_mul(
                                pb[:, :, qoff:qoff + 128],
                                pb[:, :, qoff:qoff + 128], tri_sb[:])
                        pbs.append(pb)
                    return pbs

                stage = {0: emit_scores(0)}
                if nk > 1:
                    stage[1] = emit_scores(1)
                fdone = 0
                den_put = []
                for ik in range(nk):
                    r = ik - 4 * j
                    qoff = 128 * r if r > 0 else 0
                    first, last = ik == 0, ik == nk - 1
                    pbs = stage.pop(ik)
                    for p in (0, 1):
                        for hh in (0, 1):
                            h = 2 * p + hh
                            nc.tensor.matmul(
                                pv_ps[p][64 * hh:64 * hh + 64, qoff:512],
                                vt_sb[:, ik, 64 * h:64 * h + 64],
                                pbs[p][:, hh, qoff:512],
                                start=first, stop=last,
                                tile_position=(0, 64 * hh),
                                skip_group_check=True)
                    den_put.append((ik, qoff, pbs, first, last))
                    if ik % 2 == 1 or last:
                        for dik, dqoff, dpbs, dfirst, dlast in den_put:
                            for h in range(4):
                                nc.tensor.matmul(
                                    den_ps[32 * h:32 * h + 1, dqoff:512],
                                    ones_sb[:, 0:1],
                                    dpbs[h // 2][:, h % 2, dqoff:512],
                                    start=dfirst, stop=dlast,
                                    tile_position=(0, 32 * h),
                                    skip_group_check=True)
                        den_put = []
                    want = ((ik + 1) * nf) // nk
                    while fdone < want:
                        fn, args = fillers[fdone]
                        fn(*args)
                        fdone += 1
                    if ik + 2 < nk:
                        stage[ik + 2] = emit_scores(ik + 2)
                araw = [miscp.tile([128, 512], f32, tag=f"araw{p}",
                                   name=f"araw{p}_{j}") for p in (0, 1)]
                # 1/den via the exp/ln activation table (same table set
                # as Exp -> no ACT_TABLE_LOAD): rec = exp(-ln(den)).  Keeps
                # the block-boundary chain off DVE's strict FIFO and frees
                # the den bank ~3us earlier than the exact DVE reciprocal.
                den_ln = miscp.tile([128, 512], f32, tag="densb",
                                    name=f"densb_{j}")
                rec = miscp.tile([128, 512], f16, tag="rec", name=f"rec_{j}")
                if j < NB - 1:
                    for p in (0, 1):
                        nc.vector.tensor_copy(araw[p][:], pv_ps[p][:])
                    nc.scalar.activation(den_ln[0:97, :], den_ps[0:97, :],
                                         AF.Ln)
                    nc.scalar.activation(rec[0:97, :], den_ln[0:97, :],
                                         AF.Exp, scale=-1.0)
                pending = (j, araw, rec, pv_ps, den_ps)

            # -- tail: final normalize + last output tiles.  Interleaved by
            # column half: tt 12-13 only need the first 256 qtok of block 3,
            # so their out-projection runs while half 1 normalizes. --------
            def tail_p3(tt):
                for n2 in range(2):
                    ps = scp.tile([128, 512], f32, tag="sc",
                                  name=f"p3t_{tt}_{n2}")
                    for fp in range(2):
                        nc.tensor.matmul(
                            ps[:],
                            at_sb[:, fp, tt * 128:(tt + 1) * 128],
                            wo_sb[:, fp, n2 * 512:(n2 + 1) * 512],
                            start=(fp == 0), stop=(fp == 1))
                    st = stp.tile([128, 512], f16, tag="st",
                                  name=f"stt_{tt}_{n2}")
                    if n2 == 0:
                        nc.vector.tensor_copy(st[:], ps[:])
                    else:
                        nc.scalar.copy(st[:], ps[:])
                    nc.sync.dma_start(
                        outp.ap()[tt * 128:(tt + 1) * 128,
                                  n2 * 512:(n2 + 1) * 512], st[:])

            jj, araw3, rec3, pv3, den3 = pending
            den_ln3 = miscp.tile([128, 512], f32, tag="densb", name="densb3b")
            item3 = (jj, araw3, rec3)
            for qq in range(4):
                cq = slice(128 * qq, 128 * qq + 128)
                nc.vector.tensor_copy(araw3[0][:, cq], pv3[0][:, cq])
                nc.vector.tensor_copy(araw3[1][:, cq], pv3[1][:, cq])
                nc.scalar.activation(den_ln3[0:97, cq], den3[0:97, cq], AF.Ln)
                nc.scalar.activation(rec3[0:97, cq], den_ln3[0:97, cq],
                                     AF.Exp, scale=-1.0)
                emit_norm(item3, 0, 128 * qq, 128 * qq + 128)
                emit_norm(item3, 1, 128 * qq, 128 * qq + 128)
                tail_p3(12 + qq)

    _split_sync_waits(nc)
    return nc


_NC = None


def _get_nc():
    global _NC
    if _NC is None:
        _NC = _build_nc()
    return _NC


# ---------------------------------------------------------------------------
# Host entry point
# ---------------------------------------------------------------------------

def kernel(x, qkv_w, qkv_b, out_w, out_b):
    from concourse.bass_utils import run_bass_kernel_spmd

    trace_dir = os.environ.get("BASS_KERNEL_TRACE_DIR")
    if trace_dir:
        _install_ntff_shim()

    nc = _get_nc()

    x = np.asarray(x, np.float32)
    qkv_w = np.asarray(qkv_w, np.float32)
    qkv_b = np.asarray(qkv_b, np.float32)
    out_w = np.asarray(out_w, np.float32)
    out_b = np.asarray(out_b, np.float32)

    tri_np = np.broadcast_to(np.triu(np.ones((128, 128), np.float16))[:, None, :],
        (128, 2, 128)).copy()
    in_maps = []
    for c in range(NCORES):
        b, g = divmod(c, 4)
        fs = slice(256 * g, 256 * g + 256)
        wqkvT = np.ascontiguousarray(
            np.concatenate([qkv_w[0 * E:1 * E][fs],
                            qkv_w[1 * E:2 * E][fs],
                            qkv_w[2 * E:3 * E][fs]], axis=0).T)
        bq = np.concatenate([qkv_b[0 * E:1 * E][fs],
                             qkv_b[1 * E:2 * E][fs]])[:, None]
        in_maps.append({
            "xT": np.ascontiguousarray(x[b].T).astype(np.float16),
            "wqkvT": wqkvT.astype(np.float16),
            "woT": np.ascontiguousarray(out_w[:, fs].T).astype(np.float16),
            "bqkv": np.ascontiguousarray(bq),
            "tri": tri_np,
        })

    kwargs = {}
    if trace_dir:
        kwargs = {"trace": True, "tmpdir": trace_dir}
    res = run_bass_kernel_spmd(nc, in_maps, core_ids=list(range(NCORES)), **kwargs)
    if trace_dir and res.exec_time_ns is not None:
        print(f"HW exec time: {res.exec_time_ns} ns")

    out = np.zeros((B, N, E), np.float32)
    for c in range(NCORES):
        out[c // 4] += res.results[c]["outp"].astype(np.float32)
    # v-bias is not applied on-device; its out-projection image is a constant
    # vector folded into the output bias here: (attn + b_v) @ W_o^T
    #   = attn @ W_o^T + b_v @ W_o^T.
    out += (out_b + qkv_b[2 * E:] @ out_w.T)[None, None, :]
    return out


# revision 33
# speedup vs baseline: 1.0318x; 1.0036x over previous
"""Causal self-attention (B=2, N=2048, E=1024, H=16, D=64) on 8 TRN2 cores.

Sharding: core c -> batch b = c//4, head group g = c%4 (4 heads = 256
features per core).  Each core computes its heads' q/k/v projections,
causal attention, and a partial out-projection; the host sums the 4
partials per batch.

Per-core dataflow (feature-major "transposed" layouts throughout):
  xT [E, N] (f16)  x  wqkvT slices -> q,k as [feat, tok], v as [tok, feat]
  scoresT [ktok, qtok] = k_h^T-chunks x q_h   (PE row-tiled, 2 heads/pass)
  probsT = exp(scoresT/8) in f16 (ACT), causal triangle masked (DVE)
  attnT [feat, qtok] += v-chunk^T x probsT    (PE col-tiled, 2 heads/pass)
  denom[q] += ones^T x probsT                 (PE col-tiled M=1, 4 heads)
  attnT normalized by PE-broadcast reciprocal; out = attnT^T x woT chunks.
Causal structure skips all fully-masked k-blocks (half the attention
flops); diagonal blocks are computed on their valid q-range only.

v2 perf changes vs baseline (179.3us):
  - outproj PSUM->SBUF staging copies moved off ACT (was gating exp in
    the attention-dense j=2/3 windows) onto DVE explicitly.
  - exact RECIPROCAL (3.35us on the per-block critical chain) replaced
    by reciprocal_approx_fast on PSUM + f16 cast (~1.3us chain).
  - v-bias fold: attn bias term b_v @ wo^T is added host-side into the
    output bias, removing 8 tensor_scalar_adds from the norm chain.
  - startup DMA: xT/wq chunks interleaved and triggers spread across 4
    engine queues so the first qk units start ~3us earlier.
"""

import os
import sys
import types

import numpy as np

B, N, E, H, D = 2, 2048, 1024, 16, 64
NCORES = 8


# ---------------------------------------------------------------------------
# Environment patches (this container's walrus accepts only one sync wait per
# instruction; the image's antenv lacks the NTFF profile hook shim).
# ---------------------------------------------------------------------------

def _patch_tile_drain():
    import concourse.mybir as mybir
    import concourse.tile as tile_mod
    from concourse.vector_clock import ScopedClock

    if getattr(tile_mod.TileContext, "_drain_patched", False):
        return

    def _drain_and_barrier(self, tick_clock, wait_clock):
        nc = self.nc
        probe = nc.sync.nop()
        wait_clock.add_sem_waits(probe.ins, ScopedClock({None: tick_clock.global_clock}))
        si = probe.ins.sync_info
        waits = list(si.on_wait) if si and si.on_wait else []
        if len(waits) > 1:
            si.on_wait = waits[:1]
            for w in waits[1:]:
                extra = nc.sync.nop()
                extra.ins.sync_info = mybir.SyncInfo(on_wait=[w], on_update=[])
        nc.sync.drain()
        nc.all_engine_barrier()
        assert self.sems is not None
        popped = nc._tile_sem_poison_stack.pop()
        assert popped is self._sem_poison
        nc.clear_and_free_semaphores(list(self.sems.allocated().values()))
        nc.all_engine_barrier()

    tile_mod.TileContext._drain_and_barrier = _drain_and_barrier
    tile_mod.TileContext._drain_patched = True


def _split_sync_waits(nc, max_waits=1):
    import concourse.mybir as mybir

    cnt = 0
    for f in nc.m.functions:
        for blk in f.blocks:
            insts = blk.instructions
            new = []
            for inst in insts:
                si = inst.sync_info
                waits = list(si.on_wait) if si and si.on_wait else []
                if len(waits) > max_waits:
                    keep = waits[-max_waits:]
                    excess = waits[:-max_waits]
                    for j in range(0, len(excess), max_waits):
                        n = mybir.InstNoOp(name=f"I-ws{cnt}", ins=[], outs=[])
                        cnt += 1
                        n.engine = inst.engine
                        n.sync_info = mybir.SyncInfo(
                            on_wait=excess[j:j + max_waits], on_update=[])
                        new.append(n)
                    si.on_wait = keep
                new.append(inst)
            insts[:] = new
    return cnt


def _install_ntff_shim():
    try:
        import antenv
        if "antenv.axon_hooks" in sys.modules:
            return
        mod = types.ModuleType("antenv.axon_hooks")
        mod._hook = None
        mod.set_axon_ntff_profile_hook = lambda h: setattr(mod, "_hook", h)
        mod.get_axon_ntff_profile_hook = lambda: mod._hook
        sys.modules["antenv.axon_hooks"] = mod
        antenv.axon_hooks = mod
        from trn_agent_boot.trn_boot import _ntff_profile_via_ctypes
        mod._hook = _ntff_profile_via_ctypes("/opt/axon/libaxon_pjrt.so")
    except Exception:
        pass


# ---------------------------------------------------------------------------
# Device program (identical on all 8 cores)
# ---------------------------------------------------------------------------

def _build_nc():
    import concourse.bass as bass
    import concourse.mybir as mybir
    import concourse.tile as tile

    _patch_tile_drain()

    f32 = mybir.dt.float32
    f16 = mybir.dt.float16
    AF = mybir.ActivationFunctionType

    nc = bass.Bass("TRN2", target_bir_lowering=False, debug=False)

    xT = nc.dram_tensor("xT", [E, N], f16, kind="ExternalInput")
    wqkvT = nc.dram_tensor("wqkvT", [E, 768], f16, kind="ExternalInput")
    woT = nc.dram_tensor("woT", [256, E], f16, kind="ExternalInput")
    bqkv = nc.dram_tensor("bqkv", [128, 4], f32, kind="ExternalInput")
    tri = nc.dram_tensor("tri", [128, 2, 128], f16, kind="ExternalInput")
    outp = nc.dram_tensor("outp", [N, E], f16, kind="ExternalOutput")

    NB = N // 512          # 4 token blocks of 512
    NT = N // 128          # 16 token tiles of 128
    NE = E // 128          # 8 contraction chunks
    SCALE = float(D) ** -0.5

    with nc.allow_low_precision(reason="fp16 matmul pipeline"), \
            tile.TileContext(nc) as tc:
        with tc.tile_pool(name="const", bufs=1) as constp, \
                tc.tile_pool(name="qk", bufs=1) as qkp, \
                tc.tile_pool(name="probs", bufs=6) as pbp, \
                tc.tile_pool(name="misc", bufs=2) as miscp, \
                tc.tile_pool(name="stage", bufs=4) as stp, \
                tc.tile_pool(name="mm", bufs=1, space="PSUM") as mmp, \
                tc.tile_pool(name="sc", bufs=2, space="PSUM") as scp, \
                tc.tile_pool(name="pv", bufs=1, space="PSUM") as pvp, \
                tc.tile_pool(name="den", bufs=1, space="PSUM") as denp:

            xT_sb = constp.tile([128, NE, N], f16, tag="xT")
            wq_sb = constp.tile([128, NE, 768], f16, tag="wq")
            wo_sb = constp.tile([128, 2, E], f16, tag="wo")
            bias_sb = constp.tile([128, 4, 1], f32, tag="bias")
            tri_sb = constp.tile([128, 2, 128], f16, tag="tri")
            ones_sb = constp.tile([128, 64], f16, tag="ones")
            q_sb = qkp.tile([128, 2, N], f16, tag="q")
            k_sb = qkp.tile([128, 2, N], f16, tag="k")
            vt_sb = qkp.tile([128, NT, 256], f16, tag="vt")
            at_sb = qkp.tile([128, 2, N], f16, tag="at")

            nc.vector.memset(ones_sb[:], 1.0)
            # Startup DMA: interleave xT block-0 chunks with wq chunks and
            # spread triggers over 4 engine queues so the first qk units
            # (which consume chunk e as it lands) start as early as possible.
            # Tiny constants (bias, tri) go FIRST as single descriptors: the
            # first qk unit's bias add must not wait for bulk traffic, and
            # only two trigger slots (~1.2us) delay the wq stream.
            nc.scalar.dma_start(bias_sb[:, :, 0], bqkv.ap())
            nc.scalar.dma_start(tri_sb[:], tri.ap())
            for i in range(NE):
                nc.sync.dma_start(
                    xT_sb[:, i, 0:512],
                    xT.ap()[i * 128:(i + 1) * 128, 0:512])
                nc.scalar.dma_start(
                    wq_sb[:, i, :], wqkvT.ap()[i * 128:(i + 1) * 128, :])
            for i in range(NE):
                (nc.sync if i % 2 == 0 else nc.scalar).dma_start(
                    xT_sb[:, i, 512:1024],
                    xT.ap()[i * 128:(i + 1) * 128, 512:1024])
            for i in range(NE):
                (nc.sync if i % 2 == 0 else nc.scalar).dma_start(
                    xT_sb[:, i, 1024:2048],
                    xT.ap()[i * 128:(i + 1) * 128, 1024:2048])
            for i in range(2):
                nc.scalar.dma_start(wo_sb[:, i, :], woT.ap()[i * 128:(i + 1) * 128, :])

            # -- filler work units (emitted interleaved into attention) ----
            _qk_open = {}

            def emit_qk_unit(ft, nb, pool=None, tag="mm", on_act=False,
                             half=None):
                # q/k feature tile ft (0,1=q; 2,3=k), token block nb.
                # on_act: drain the PSUM bank via ACT instead of DVE, for
                # fillers adjacent to the block-boundary reciprocal chain
                # (DVE strict FIFO would park the bank behind the recips).
                # half=0/1: emit only e-chunks 0-3 / 4-7 (finer filler
                # pacing); half 0 allocates the bank, half 1 finishes+drains.
                if half == 0:
                    ps = (pool or mmp).tile([128, 512], f32, tag=tag,
                                            name=f"qk_{ft}_{nb}")
                    _qk_open[(ft, nb)] = ps
                    erange = range(0, NE // 2)
                elif half == 1:
                    ps = _qk_open.pop((ft, nb))
                    erange = range(NE // 2, NE)
                else:
                    ps = (pool or mmp).tile([128, 512], f32, tag=tag,
                                            name=f"qk_{ft}_{nb}")
                    erange = range(NE)
                for e in erange:
                    nc.tensor.matmul(
                        ps[:], wq_sb[:, e, ft * 128:(ft + 1) * 128],
                        xT_sb[:, e, nb * 512:(nb + 1) * 512],
                        start=(e == 0), stop=(e == NE - 1))
                if half == 0:
                    return
                dest = (q_sb if ft < 2 else k_sb)[:, ft % 2,
                                                  nb * 512:(nb + 1) * 512]
                if on_act:
                    nc.scalar.add(dest, ps[:], bias_sb[:, ft, :])
                else:
                    nc.vector.tensor_scalar_add(dest, ps[:], bias_sb[:, ft, :])

            def emit_v_unit(tt, pool=None, tag="mm", on_act=False):
                ps = (pool or mmp).tile([128, 256], f32, tag=tag,
                                        name=f"v_{tt}")
                for e in range(NE):
                    nc.tensor.matmul(
                        ps[:], xT_sb[:, e, tt * 128:(tt + 1) * 128],
                        wq_sb[:, e, 512:768],
                        start=(e == 0), stop=(e == NE - 1))
                if on_act:
                    nc.scalar.copy(vt_sb[:, tt, :], ps[:])
                else:
                    nc.vector.tensor_copy(vt_sb[:, tt, :], ps[:])

            def emit_p3_unit(tt, on_act=False):
                for n2 in range(2):
                    ps = mmp.tile([128, 512], f32, tag="mm",
                                  name=f"p3_{tt}_{n2}")
                    for fp in range(2):
                        nc.tensor.matmul(
                            ps[:],
                            at_sb[:, fp, tt * 128:(tt + 1) * 128],
                            wo_sb[:, fp, n2 * 512:(n2 + 1) * 512],
                            start=(fp == 0), stop=(fp == 1))
                    st = stp.tile([128, 512], f16, tag="st",
                                  name=f"st_{tt}_{n2}")
                    if on_act:
                        nc.scalar.copy(st[:], ps[:])
                    else:
                        nc.vector.tensor_copy(st[:], ps[:])
                    nc.sync.dma_start(
                        outp.ap()[tt * 128:(tt + 1) * 128,
                                  n2 * 512:(n2 + 1) * 512], st[:])

            def emit_norm(item, p, c0, c1):
                # PE broadcast of 1/denom + normalize into attnT for head
                # pair p, qtok columns [c0, c1) of the block.  The two
                # matmuls hit disjoint partition halves of one PSUM bank, so
                # they run concurrently without a bank collision; the DVE
                # multiply latency is covered by fillers emitted in between
                # successive emit_norm calls.
                jj, araw, rec = item
                w = c1 - c0
                cs = slice(c0, c1)
                js = slice(512 * jj + c0, 512 * jj + c1)
                bc = mmp.tile([128, 512], f32, tag="mm",
                              name=f"bc_{jj}_{p}_{c0}")
                for hh in (0, 1):
                    h = 2 * p + hh
                    nc.tensor.matmul(
                        bc[64 * hh:64 * hh + 64, 0:w],
                        ones_sb[32 * h:32 * h + 1, 0:64],
                        rec[32 * h:32 * h + 1, cs],
                        start=True, stop=True,
                        tile_position=(32 * h, 64 * hh))
                nc.vector.tensor_mul(at_sb[:, p, js],
                                     araw[p][:, cs], bc[:, 0:w])

            # -- p1 block 0 up front (attention j=0 needs it).  Attention
            # PSUM banks are idle here; round-robin the units through them
            # so consecutive units don't serialize on one bank's DVE drain.
            upools = [(mmp, "mm"), (pvp, "pv0"), (pvp, "pv1"), (denp, "den")]
            for u, ft in enumerate(range(4)):
                emit_qk_unit(ft, 0, *upools[u % 4])
            for u, tt in enumerate(range(4)):
                emit_v_unit(tt, *upools[u % 4])

            # -- attention blocks: depth-2 software pipeline ---------------
            # scores/exp for k-block ik+2 are emitted while pv/den of block
            # ik execute, so ACT (exp) and PE overlap instead of ping-pong.
            pending = None
            for j in range(NB):
                if j == 0:
                    fillers = []
                    for ft in range(4):
                        fillers += [(emit_qk_unit, (ft, 1, None, "mm", False, 0)),
                                    (emit_qk_unit, (ft, 1, None, "mm", False, 1))]
                    fillers += [(emit_v_unit, (tt,)) for tt in range(4, 8)]
                elif j == 1:
                    fillers = []
                    for ft in range(4):
                        fillers += [(emit_qk_unit, (ft, 2, None, "mm", False, 0)),
                                    (emit_qk_unit, (ft, 2, None, "mm", False, 1))]
                    fillers += [(emit_v_unit, (tt,)) for tt in range(8, 12)]
                elif j == 2:
                    fillers = []
                    for ft in range(4):
                        fillers += [(emit_qk_unit, (ft, 3, None, "mm", False, 0)),
                                    (emit_qk_unit, (ft, 3, None, "mm", False, 1))]
                    fillers += [(emit_p3_unit, (tt,)) for tt in range(2)]
                else:
                    fillers = [(emit_v_unit, (tt,)) for tt in range(12, 16)]
                    fillers += [(emit_p3_unit, (tt,)) for tt in range(2, 12)]
                if pending is not None:
                    fillers.insert(min(3, len(fillers)),
                                   (emit_norm, (pending[:3], 0, 0, 512)))
                    fillers.insert(min(5, len(fillers)),
                                   (emit_norm, (pending[:3], 1, 0, 512)))
                    pending = None
                nf = len(fillers)
                pv_ps = [pvp.tile([128, 512], f32, tag=f"pv{p}",
                                  name=f"pv{p}_{j}") for p in (0, 1)]
                den_ps = denp.tile([128, 512], f32, tag="den",
                                   name=f"den_{j}")
                nk = 4 * (j + 1)

                def emit_scores(ik, j=j):
                    r = ik - 4 * j
                    qoff = 128 * r if r > 0 else 0
                    qs = slice(512 * j + qoff, 512 * (j + 1))
                    pbs = []
                    for p in (0, 1):
                        sc = scp.tile([128, 2, 512], f32, tag="sc",
                                      name=f"sc_{j}_{ik}_{p}")
                        for hh in (0, 1):
                            dsl = slice(64 * hh, 64 * hh + 64)
                            nc.tensor.matmul(
                                sc[:, hh, qoff:512],
                                k_sb[dsl, p, ik * 128:(ik + 1) * 128],
                                q_sb[dsl, p, qs],
                                start=True, stop=True)
                        pb = pbp.tile([128, 2, 512], f16, tag="pb",
                                      name=f"pb_{j}_{ik}_{p}")
                        nc.scalar.activation(pb[:, :, qoff:512],
                                             sc[:, :, qoff:512],
                                             AF.Exp, scale=SCALE)
                        if r >= 0:
                            nc.gpsimd.tensor_mul(
                                pb[:, :, qoff:qoff + 128],
                                pb[:, :, qoff:qoff + 128], tri_sb[:])
                        pbs.append(pb)
                    return pbs

                stage = {0: emit_scores(0)}
                if nk > 1:
                    stage[1] = emit_scores(1)
                fdone = 0
                den_put = []
                for ik in range(nk):
                    r = ik - 4 * j
                    qoff = 128 * r if r > 0 else 0
                    first, last = ik == 0, ik == nk - 1
                    pbs = stage.pop(ik)
                    for p in (0, 1):
                        for hh in (0, 1):
                            h = 2 * p + hh
                            nc.tensor.matmul(
                                pv_ps[p][64 * hh:64 * hh + 64, qoff:512],
                                vt_sb[:, ik, 64 * h:64 * h + 64],
                                pbs[p][:, hh, qoff:512],
                                start=first, stop=last,
                                tile_position=(0, 64 * hh),
                                skip_group_check=True)
                    den_put.append((ik, qoff, pbs, first, last))
                    if ik % 2 == 1 or last:
                        for dik, dqoff, dpbs, dfirst, dlast in den_put:
                            for h in range(4):
                                nc.tensor.matmul(
                                    den_ps[32 * h:32 * h + 1, dqoff:512],
                                    ones_sb[:, 0:1],
                                    dpbs[h // 2][:, h % 2, dqoff:512],
                                    start=dfirst, stop=dlast,
                                    tile_position=(0, 32 * h),
                                    skip_group_check=True)
                        den_put = []
                    want = ((ik + 1) * nf) // nk
                    while fdone < want:
                        fn, args = fillers[fdone]
                        fn(*args)
                        fdone += 1
                    if ik + 2 < nk:
                        stage[ik + 2] = emit_scores(ik + 2)
                araw = [miscp.tile([128, 512], f32, tag=f"araw{p}",
                                   name=f"araw{p}_{j}") for p in (0, 1)]
                # 1/den via the exp/ln activation table (same table set
                # as Exp -> no ACT_TABLE_LOAD): rec = exp(-ln(den)).  Keeps
                # the block-boundary chain off DVE's strict FIFO and frees
                # the den bank ~3us earlier than the exact DVE reciprocal.
                den_ln = miscp.tile([128, 512], f32, tag="densb",
                                    name=f"densb_{j}")
                rec = miscp.tile([128, 512], f16, tag="rec", name=f"rec_{j}")
                if j < NB - 1:
                    for p in (0, 1):
                        nc.vector.tensor_copy(araw[p][:], pv_ps[p][:])
                    nc.scalar.activation(den_ln[0:97, :], den_ps[0:97, :],
                                         AF.Ln)
                    nc.scalar.activation(rec[0:97, :], den_ln[0:97, :],
                                         AF.Exp, scale=-1.0)
                pending = (j, araw, rec, pv_ps, den_ps)

            # -- tail: final normalize + last output tiles.  Interleaved by
            # column half: tt 12-13 only need the first 256 qtok of block 3,
            # so their out-projection runs while half 1 normalizes. --------
            def tail_p3(tt):
                for n2 in range(2):
                    ps = scp.tile([128, 512], f32, tag="sc",
                                  name=f"p3t_{tt}_{n2}")
                    for fp in range(2):
                        nc.tensor.matmul(
                            ps[:],
                            at_sb[:, fp, tt * 128:(tt + 1) * 128],
                            wo_sb[:, fp, n2 * 512:(n2 + 1) * 512],
                            start=(fp == 0), stop=(fp == 1))
                    st = stp.tile([128, 512], f16, tag="st",
                                  name=f"stt_{tt}_{n2}")
                    if n2 == 0:
                        nc.vector.tensor_copy(st[:], ps[:])
                    else:
                        nc.scalar.copy(st[:], ps[:])
                    nc.sync.dma_start(
                        outp.ap()[tt * 128:(tt + 1) * 128,
                                  n2 * 512:(n2 + 1) * 512], st[:])

            jj, araw3, rec3, pv3, den3 = pending
            den_ln3 = miscp.tile([128, 512], f32, tag="densb", name="densb3b")
            item3 = (jj, araw3, rec3)
            for qq in range(4):
                cq = slice(128 * qq, 128 * qq + 128)
                nc.vector.tensor_copy(araw3[0][:, cq], pv3[0][:, cq])
                nc.vector.tensor_copy(araw3[1][:, cq], pv3[1][:, cq])
                nc.scalar.activation(den_ln3[0:97, cq], den3[0:97, cq], AF.Ln)
                nc.scalar.activation(rec3[0:97, cq], den_ln3[0:97, cq],
                                     AF.Exp, scale=-1.0)
                emit_norm(item3, 0, 128 * qq, 128 * qq + 128)
                emit_norm(item3, 1, 128 * qq, 128 * qq + 128)
                tail_p3(12 + qq)

    _split_sync_waits(nc)
    return nc


_NC = None


def _get_nc():
    global _NC
    if _NC is None:
        _NC = _build_nc()
    return _NC


# ---------------------------------------------------------------------------
# Host entry point
# ---------------------------------------------------------------------------

def kernel(x, qkv_w, qkv_b, out_w, out_b):
    from concourse.bass_utils import run_bass_kernel_spmd

    trace_dir = os.environ.get("BASS_KERNEL_TRACE_DIR")
    if trace_dir:
        _install_ntff_shim()

    nc = _get_nc()

    x = np.asarray(x, np.float32)
    qkv_w = np.asarray(qkv_w, np.float32)
    qkv_b = np.asarray(qkv_b, np.float32)
    out_w = np.asarray(out_w, np.float32)
    out_b = np.asarray(out_b, np.float32)

    tri_np = np.broadcast_to(np.triu(np.ones((128, 128), np.float16))[:, None, :],
        (128, 2, 128)).copy()
    in_maps = []
    for c in range(NCORES):
        b, g = divmod(c, 4)
        fs = slice(256 * g, 256 * g + 256)
        wqkvT = np.ascontiguousarray(
            np.concatenate([qkv_w[0 * E:1 * E][fs],
                            qkv_w[1 * E:2 * E][fs],
                            qkv_w[2 * E:3 * E][fs]], axis=0).T)
        bq = np.concatenate([qkv_b[0 * E:1 * E][fs],
                             qkv_b[1 * E:2 * E][fs]])[:, None]
        in_maps.append({
            "xT": np.ascontiguousarray(x[b].T).astype(np.float16),
            "wqkvT": wqkvT.astype(np.float16),
            "woT": np.ascontiguousarray(out_w[:, fs].T).astype(np.float16),
            "bqkv": np.ascontiguousarray(bq),
            "tri": tri_np,
        })

    kwargs = {}
    if trace_dir:
        kwargs = {"trace": True, "tmpdir": trace_dir}
    res = run_bass_kernel_spmd(nc, in_maps, core_ids=list(range(NCORES)), **kwargs)
    if trace_dir and res.exec_time_ns is not None:
        print(f"HW exec time: {res.exec_time_ns} ns")

    out = np.zeros((B, N, E), np.float32)
    for c in range(NCORES):
        out[c // 4] += res.results[c]["outp"].astype(np.float32)
    # v-bias is not applied on-device; its out-projection image is a constant
    # vector folded into the output bias here: (attn + b_v) @ W_o^T
    #   = attn @ W_o^T + b_v @ W_o^T.
    out += (out_b + qkv_b[2 * E:] @ out_w.T)[None, None, :]
    return out
